# revision 1
# baseline (speedup 1.0000x reference)
"""Trainium2 Bass kernel: multi-head attention (B=2, S=2048, E=1024, H=16).

Sharding: 8 cores = 2 batches x 4 head-groups. Core c handles batch c//4 and
heads [4*(c%4), 4*(c%4)+4) (256 feature columns of the projections).

Per-core device program (all matmuls in fp32r):
  - inputs: xT [E,S] (host-transposed x[b]), wqT/wkT/wvT [E,256] (host-
    transposed row-slices of Wq/Wk/Wv), woT [256,E] (host-transposed column
    slice of Wo).
  - qT,kT [256,S] = (x @ W^T)^T per head-group, computed directly in [f,s]
    layout; v [S,256] in [s,f] layout with a ones column appended per head.
  - per (head, qi-chunk): scores^T tiles [128 kj, 512 qi] on PE, exp on ACT
    (sm_scale folded into the activation scale), attn@v accumulated on PE with
    the ones column producing the softmax denominator in partition 64,
    then reciprocal + GPSIMD partition-broadcast + multiply to normalize;
    output kept in [f, s] layout for the output projection.
  - out_partial [S,E] = o^T^T @ Wo^T column-slice; host sums 4 partials per
    batch and adds bo.
"""

import numpy as np

import concourse.tile as tile
import concourse.mybir as mybir
from concourse import bacc
from concourse.bass_utils import run_bass_kernel_spmd

B, S, E, H, D = 2, 2048, 1024, 16, 64
NCORES = 8
GPB = NCORES // B      # head-groups (cores) per batch = 4
HPC = H // GPB         # heads per core = 4
FPC = HPC * D          # feature cols per core = 256
SM = float(D) ** -0.5  # softmax scale

F32 = mybir.dt.float32
F32R = mybir.dt.float32r

P = 128
NE = E // P            # 8 e-tiles
NST = S // P           # 16 s-tiles (key tiles)
NQ = 4                 # qi chunks
QC = S // NQ           # 512
KTG = 2                # k-tiles per psum/exp group
NKG = NST // KTG       # 8 groups
FT = FPC // P          # 2 f-tiles per core


def _round_fp32r(a: np.ndarray) -> np.ndarray:
    """Round fp32 to the fp32r encoding (RNE to 12-bit mantissa)."""
    u = np.ascontiguousarray(a, dtype=np.float32).view(np.uint32)
    lo = u & np.uint32(0xFFF)
    base = u & ~np.uint32(0xFFF)
    rup = (lo > 0x800) | ((lo == 0x800) & (((base >> np.uint32(12)) & np.uint32(1)) == 1))
    out = base + (rup.astype(np.uint32) << np.uint32(12))
    return out.view(np.float32)


def _build():
    nc = bacc.Bacc("TRN2", target_bir_lowering=False, debug=False)

    xT_d = nc.dram_tensor("xT", [E, S], F32R, kind="ExternalInput")
    wq_d = nc.dram_tensor("wqT", [E, FPC], F32R, kind="ExternalInput")
    wk_d = nc.dram_tensor("wkT", [E, FPC], F32R, kind="ExternalInput")
    wv_d = nc.dram_tensor("wvT", [E, FPC], F32R, kind="ExternalInput")
    wo_d = nc.dram_tensor("woT", [FPC, E], F32R, kind="ExternalInput")
    ones_lhs_d = nc.dram_tensor("ones_lhs", [1, D], F32R, kind="ExternalInput")
    ones_col_d = nc.dram_tensor("ones_col", [P, HPC, 1], F32R, kind="ExternalInput")
    out_d = nc.dram_tensor("out", [S, E], F32, kind="ExternalOutput")

    with tile.TileContext(nc) as tc:
        with (
            tc.tile_pool(name="wpool", bufs=1) as wpool,
            tc.tile_pool(name="xpool", bufs=1) as xpool,
            tc.tile_pool(name="qkpool", bufs=1) as qkpool,
            tc.tile_pool(name="vpool", bufs=1) as vpool,
            tc.tile_pool(name="opool", bufs=1) as opool,
            tc.tile_pool(name="epool", bufs=3) as epool,
            tc.tile_pool(name="spool", bufs=2) as spool,
            tc.tile_pool(name="outpool", bufs=3) as outpool,
            tc.tile_pool(name="pspool", bufs=2, space="PSUM") as pspool,
            tc.tile_pool(name="popool", bufs=2, space="PSUM") as popool,
            tc.tile_pool(name="oaccpool", bufs=2, space="PSUM") as oaccpool,
        ):
            # ---- weights / constants -------------------------------------
            wq = wpool.tile([P, NE, FPC], F32R, name="wq")
            wk = wpool.tile([P, NE, FPC], F32R, name="wk")
            wv = wpool.tile([P, NE, FPC], F32R, name="wv")
            wo = wpool.tile([P, FT, E], F32R, name="wo")
            ones = wpool.tile([1, D], F32R, name="ones")
            wk_r = wk_d.ap().rearrange("(t p) f -> p t f", p=P)
            wq_r = wq_d.ap().rearrange("(t p) f -> p t f", p=P)
            # f-tile-0 halves first: only they gate the first score matmuls;
            # the ft1 halves ride behind the early x chunks.
            nc.sync.dma_start(out=wk[:, :, 0:P], in_=wk_r[:, :, 0:P])
            nc.sync.dma_start(out=wq[:, :, 0:P], in_=wq_r[:, :, 0:P])

            # ---- x^T (chunk-major DMA so compute starts early) -----------
            xT_r = xT_d.ap().rearrange("(t p) s -> p t s", p=P)
            xts = [
                xpool.tile([P, S], F32R, name=f"xt{et}", tag=f"xt{et}")
                for et in range(NE)
            ]
            for cq in range(NQ):
                csl = slice(cq * QC, (cq + 1) * QC)
                for et in range(NE):
                    nc.sync.dma_start(out=xts[et][:, csl], in_=xT_r[:, et, csl])
                if cq == 0:
                    nc.sync.dma_start(
                        out=wv, in_=wv_d.ap().rearrange("(t p) f -> p t f", p=P)
                    )
                    nc.sync.dma_start(out=ones, in_=ones_lhs_d.ap())
                elif cq == 2:
                    nc.sync.dma_start(out=wk[:, :, P:FPC], in_=wk_r[:, :, P:FPC])
                    nc.sync.dma_start(out=wq[:, :, P:FPC], in_=wq_r[:, :, P:FPC])

            nc.sync.dma_start(out=wo, in_=wo_d.ap().rearrange("(t p) g -> p t g", p=P))

            # ---- v projection: v[s, f] with ones col per head ------------
            v_tiles = [
                vpool.tile([P, HPC, D + 1], F32R, name=f"v{st}", tag=f"v{st}")
                for st in range(NST)
            ]

            def proj_v(st):
                vt = v_tiles[st]
                nc.sync.dma_start(out=vt[:, :, D : D + 1], in_=ones_col_d.ap())
                ps_v = popool.tile([P, FPC], F32, name="ps_v", tag="po")
                for et in range(NE):
                    nc.tensor.matmul(
                        ps_v,
                        xts[et][:, st * P : (st + 1) * P],
                        wv[:, et, :],
                        start=(et == 0),
                        stop=(et == NE - 1),
                    )
                nc.vector.tensor_copy(
                    vt[:, :, 0:D], ps_v.rearrange("p (h d) -> p h d", d=D)
                )

            # ---- q^T / k^T projections: [f, s] ---------------------------
            def proj_T(w_tile, dst_tiles, which, ft, cq):
                ps = popool.tile([P, QC], F32, name=f"ps_{which}", tag="po")
                for et in range(NE):
                    nc.tensor.matmul(
                        ps,
                        w_tile[:, et, ft * P : (ft + 1) * P],
                        xts[et][:, cq * QC : (cq + 1) * QC],
                        start=(et == 0),
                        stop=(et == NE - 1),
                    )
                nc.vector.tensor_copy(
                    dst_tiles[ft][:, cq * QC : (cq + 1) * QC], ps
                )

            kts = [qkpool.tile([P, S], F32R, name=f"kt{ft}", tag=f"kt{ft}") for ft in range(FT)]
            qts = [qkpool.tile([P, S], F32R, name=f"qt{ft}", tag=f"qt{ft}") for ft in range(FT)]
            ots = [opool.tile([P, S], F32R, name=f"ot{ft}", tag=f"ot{ft}") for ft in range(FT)]

            # Filler machinery: generators that emit one PE-side instruction
            # per next() call. attn_core drains a couple of units after each
            # kt step, so independent matmul work lands inside the PE idle
            # gaps of the ACT-bound attention inner loop instead of between
            # cores (the PE executes its stream in order).
            from collections import deque

            fillers = deque()

            def pump(n):
                for _ in range(n):
                    while fillers:
                        try:
                            next(fillers[0])
                            break
                        except StopIteration:
                            fillers.popleft()
                    else:
                        return

            def attn_core(pair, cq, per_kt=2):
                """Heads 2*pair, 2*pair+1 for query chunk cq; the two heads'
                score matmuls run concurrently on PE row-groups 0-63/64-127.
                Returns the two accumulation psum tiles (rows 0..63 =
                sum(exp*v), row 64 = sum(exp))."""
                ft = pair
                csl = slice(cq * QC, (cq + 1) * QC)
                ps_o = [
                    oaccpool.tile([D + 1, QC], F32, name=f"ps_o{s}", tag="oacc")
                    for s in range(2)
                ]
                for kt in range(NST):
                    et_t = epool.tile([P, 2, QC], F32R, name="et_t", tag="et_t")
                    ps_s = pspool.tile([P, 2, QC], F32, name="ps_s", tag="ps_s")
                    for sub in range(2):
                        lo, hi = sub * D, (sub + 1) * D
                        nc.tensor.matmul(
                            ps_s[:, sub, :],
                            kts[ft][lo:hi, kt * P : (kt + 1) * P],
                            qts[ft][lo:hi, csl],
                            start=True,
                            stop=True,
                        )
                    nc.scalar.activation(
                        out=et_t,
                        in_=ps_s,
                        func=mybir.ActivationFunctionType.Exp,
                        scale=SM,
                    )
                    for sub in range(2):
                        nc.tensor.matmul(
                            ps_o[sub],
                            v_tiles[kt][:, 2 * pair + sub, :],
                            et_t[:, sub, :],
                            start=(kt == 0),
                            stop=(kt == NST - 1),
                        )
                    if kt > 0:
                        pump(per_kt)
                return ps_o

            def attn_drain(ps_o):
                """Copy both accumulators (incl. the sum row) to SBUF right
                away so the psum slots free early."""
                o_full = []
                for sub in range(2):
                    of = epool.tile([D + 1, QC], F32, name="o_hat", tag="o_hat", bufs=4)
                    nc.vector.tensor_copy(of, ps_o[sub])
                    o_full.append(of)
                return o_full

            def bcast_recip(o_full):
                """Reciprocal of each sum row, partition-broadcast on the
                (otherwise idle) GPSIMD engine. No PE/ACT work."""
                bcs = []
                for sub in range(2):
                    rec = spool.tile([1, QC], F32, name="rec", tag="rec", bufs=1)
                    nc.vector.reciprocal(rec, o_full[sub][D : D + 1, :])
                    bc = spool.tile([D, QC], F32, name="bc", tag="bc", bufs=4)
                    nc.gpsimd.partition_broadcast(bc, rec)
                    bcs.append(bc)
                return bcs

            def attn_finish(pair, cq, o_full):
                """Normalize a pair-0 chunk (full-width multiply)."""
                csl = slice(cq * QC, (cq + 1) * QC)
                bcs = bcast_recip(o_full)
                for sub in range(2):
                    lo, hi = sub * D, (sub + 1) * D
                    nc.vector.tensor_mul(
                        ots[pair][lo:hi, csl], o_full[sub][0:D, :], bcs[sub]
                    )

            def finish_outproj_units(cq, o_full, bcs, tail=False):
                """Pair-1 normalize pipelined with the output projection at
                s-tile granularity (shortens the kernel tail). In the tail
                the PSUM->SBUF copies ride the idle ACT engine instead of
                DVE."""
                for sti in range(NQ):
                    st = cq * NQ + sti
                    ssl = slice(sti * P, (sti + 1) * P)
                    for sub in range(2):
                        lo, hi = sub * D, (sub + 1) * D
                        nc.vector.tensor_mul(
                            ots[1][lo:hi, st * P : (st + 1) * P],
                            o_full[sub][0:D, ssl],
                            bcs[sub][:, ssl],
                        )
                    yield
                    out_sb = outpool.tile([P, E], F32, name="out_sb", tag="out_sb")
                    for gc in range(2):
                        ps_out = popool.tile([P, QC], F32, name="ps_out", tag="po")
                        for ft in range(FT):
                            nc.tensor.matmul(
                                ps_out,
                                ots[ft][:, st * P : (st + 1) * P],
                                wo[:, ft, gc * QC : (gc + 1) * QC],
                                start=(ft == 0),
                                stop=(ft == FT - 1),
                            )
                            yield
                        if tail:
                            nc.scalar.activation(
                                out=out_sb[:, gc * QC : (gc + 1) * QC],
                                in_=ps_out,
                                func=mybir.ActivationFunctionType.Copy,
                            )
                        else:
                            nc.vector.tensor_copy(
                                out_sb[:, gc * QC : (gc + 1) * QC], ps_out
                            )
                        yield
                    nc.sync.dma_start(
                        out=out_d.ap()[st * P : (st + 1) * P, :], in_=out_sb
                    )

            # Emission order = scheduler priority. Attention cores are
            # emitted right after the projections of their own chunk, so the
            # first exp fires as soon as chunk-0 data exists; later-chunk
            # projections backfill PE whenever attention is dep-blocked.
            def proj1_units():
                for cq in range(NQ):
                    for w_tile, dst, which in ((wk, kts, "k1"), (wq, qts, "q1")):
                        ps = popool.tile([P, QC], F32, name=f"ps_{which}", tag="po")
                        for et in range(NE):
                            nc.tensor.matmul(
                                ps,
                                w_tile[:, et, P : 2 * P],
                                xts[et][:, cq * QC : (cq + 1) * QC],
                                start=(et == 0),
                                stop=(et == NE - 1),
                            )
                            yield
                        nc.vector.tensor_copy(
                            dst[1][:, cq * QC : (cq + 1) * QC], ps
                        )
                        yield

            for cq in range(NQ):
                proj_T(wk, kts, "k0", 0, cq)
                proj_T(wq, qts, "q0", 0, cq)
                for st in range(cq * NQ, (cq + 1) * NQ):
                    proj_v(st)

            PER_KT = {(0, 1): 2}
            for pair in range(2):
                for cq in range(NQ):
                    ps_o = attn_core(pair, cq, per_kt=PER_KT.get((pair, cq), 2 if pair else 1))
                    of = attn_drain(ps_o)
                    if pair == 0:
                        attn_finish(pair, cq, of)
                    elif cq < NQ - 1:
                        bcs = bcast_recip(of)
                        fillers.append(finish_outproj_units(cq, of, bcs))
                    else:
                        # tail chunk: broadcast via a PE matmul (shortest
                        # latency chain right after the last core)
                        bcs = []
                        for sub in range(2):
                            rec = spool.tile([1, QC], F32, name="rec", tag="rec", bufs=1)
                            nc.vector.reciprocal(rec, of[sub][D : D + 1, :])
                            rec_r = spool.tile([1, QC], F32R, name="rec_r", tag="rec_r", bufs=1)
                            nc.vector.tensor_copy(rec_r, rec)
                            ps_bc = popool.tile([D, QC], F32, name="ps_bc", tag="po")
                            nc.tensor.matmul(ps_bc, ones, rec_r, start=True, stop=True)
                            bcs.append(ps_bc)
                        fillers.appendleft(
                            finish_outproj_units(cq, of, bcs, tail=True)
                        )
                    if pair == 0 and cq == 0:
                        fillers.append(proj1_units())
            # drain remaining fillers (the last chunk's output projection)
            while fillers:
                pump(64)

    nc.compile()
    return nc


_NC_CACHE = None


def _get_nc():
    global _NC_CACHE
    if _NC_CACHE is None:
        _NC_CACHE = _build()
    return _NC_CACHE


def make_in_maps(x, Wq, Wk, Wv, Wo):
    in_maps = []
    xTs = [_round_fp32r(x[b].T) for b in range(B)]
    for c in range(NCORES):
        b, hg = c // GPB, c % GPB
        fsl = slice(hg * FPC, (hg + 1) * FPC)
        in_maps.append({
            "xT": xTs[b],
            "wqT": _round_fp32r(Wq[fsl, :].T),
            "wkT": _round_fp32r(Wk[fsl, :].T),
            "wvT": _round_fp32r(Wv[fsl, :].T),
            "woT": _round_fp32r(Wo[:, fsl].T),
            "ones_lhs": np.ones((1, D), dtype=np.float32),
            "ones_col": np.ones((P, HPC, 1), dtype=np.float32),
        })
    return in_maps


def kernel(x, Wq, bq, Wk, bk, Wv, bv, Wo, bo):
    x = np.asarray(x, dtype=np.float32)
    Wq, Wk, Wv, Wo = (np.asarray(a, dtype=np.float32) for a in (Wq, Wk, Wv, Wo))
    bq, bk, bv, bo = (np.asarray(a, dtype=np.float32) for a in (bq, bk, bv, bo))
    if np.any(bq) or np.any(bk) or np.any(bv):
        # fall back: fold nonzero projection biases into an augmented input
        # row is not implemented; biases are zero for this problem spec.
        raise NotImplementedError("nonzero projection biases not supported")

    nc = _get_nc()
    in_maps = make_in_maps(x, Wq, Wk, Wv, Wo)
    res = run_bass_kernel_spmd(nc, in_maps, core_ids=list(range(NCORES)))
    out = np.empty((B, S, E), dtype=np.float32)
    for b in range(B):
        acc = res.results[b * GPB]["out"].astype(np.float32).copy()
        for hg in range(1, GPB):
            acc += res.results[b * GPB + hg]["out"]
        out[b] = acc
    out += bo[None, None, :]
    return out



# revision 4
# speedup vs baseline: 1.1805x; 1.1805x over previous
"""Trainium2 Bass kernel: multi-head attention (B=2, S=2048, E=1024, H=16).

Sharding: 8 cores = 2 batches x 4 head-groups. Core c handles batch c//4 and
heads [4*(c%4), 4*(c%4)+4) (256 feature columns of the projections).

Per-core device program (fp16 operands, fp32 psum accumulation):
  - inputs: xT [E,S] (host-transposed x[b]), wqT/wkT/wvT [E,256] (host-
    transposed row-slices of Wq/Wk/Wv), woT [256,E] (host-transposed column
    slice of Wo), ident [128,128] identity for PE transposes.
  - qT,kT [256,S] = (x @ W^T)^T per head-group in [f,s] layout; v [S,256] in
    [s,f] layout with a ones column appended per head (softmax denominator).
  - per (head-pair, qi-chunk): scores^T tiles [128 kj, 512 qi] on PE, exp on
    ACT (sm_scale folded into the activation scale) to fp16; attn@v in the
    flipped orientation out[q, d] = et[k, q]^T @ v[k, d+1] streaming only 65
    rows per matmul (half the PE cost of streaming queries), accumulated over
    the 16 k-tiles; col 64 of the accumulator is the softmax denominator.
  - normalize fused into psum evacuation: per-partition reciprocal multiply
    (tensor_scalar with [P,1] scalar AP) -> o [s, f] fp16; PE transpose with
    the identity -> o^T [f, s] for the output projection.
  - out_partial [S,E] fp16 = o^T^T @ Wo^T column-slice; host sums 4 partials
    per batch in fp32 and adds bo.

Schedule: emission order = Tile scheduler priority. Chunk-0 projections are
emitted first so the first exp fires ~7us in; every later projection, the
normalize/transpose drains, and the output projections are filler generators
pumped into the PE idle gaps of the ACT-bound attention inner loop.
"""

from collections import deque

import numpy as np

import concourse.tile as tile
import concourse.mybir as mybir
from concourse import bacc
from concourse.bass_utils import run_bass_kernel_spmd

B, S, E, H, D = 2, 2048, 1024, 16, 64
NCORES = 8
GPB = NCORES // B      # head-groups (cores) per batch = 4
HPC = H // GPB         # heads per core = 4
FPC = HPC * D          # feature cols per core = 256
SM = float(D) ** -0.5  # softmax scale

F32 = mybir.dt.float32
F16 = mybir.dt.float16

P = 128
NE = E // P            # 8 e-tiles
NST = S // P           # 16 s-tiles (key tiles)
NQ = 4                 # qi chunks
QC = S // NQ           # 512
NQT = QC // P          # 4 q-tiles per chunk
FT = FPC // P          # 2 f-tiles per core


def _build():
    nc = bacc.Bacc("TRN2", target_bir_lowering=False, debug=False)

    xT_d = nc.dram_tensor("xT", [E, S], F16, kind="ExternalInput")
    wq_d = nc.dram_tensor("wqT", [E, FPC], F16, kind="ExternalInput")
    wk_d = nc.dram_tensor("wkT", [E, FPC], F16, kind="ExternalInput")
    wv_d = nc.dram_tensor("wvT", [E, FPC], F16, kind="ExternalInput")
    wo_d = nc.dram_tensor("woT", [FPC, E], F16, kind="ExternalInput")
    id_d = nc.dram_tensor("ident", [P, P], F16, kind="ExternalInput")
    out_d = nc.dram_tensor("out", [S, E], F16, kind="ExternalOutput")

    with tile.TileContext(nc) as tc:
        with (
            tc.tile_pool(name="wpool", bufs=1) as wpool,
            tc.tile_pool(name="xpool", bufs=1) as xpool,
            tc.tile_pool(name="qkpool", bufs=1) as qkpool,
            tc.tile_pool(name="vpool", bufs=1) as vpool,
            tc.tile_pool(name="opool", bufs=1) as opool,
            tc.tile_pool(name="epool", bufs=4) as epool,
            tc.tile_pool(name="spool", bufs=2) as spool,
            tc.tile_pool(name="outpool", bufs=3) as outpool,
            tc.tile_pool(name="pspool", bufs=2, space="PSUM") as pspool,
            tc.tile_pool(name="popool", bufs=2, space="PSUM") as popool,
            tc.tile_pool(name="oaccpool", bufs=2, space="PSUM") as oaccpool,
        ):
            # ---- weights / constants -------------------------------------
            wq = wpool.tile([P, NE, FPC], F16, name="wq")
            wk = wpool.tile([P, NE, FPC], F16, name="wk")
            wv = wpool.tile([P, NE, FPC], F16, name="wv")
            wo = wpool.tile([P, FT, E], F16, name="wo")
            ident = wpool.tile([P, P], F16, name="ident")
            nc.sync.dma_start(out=wk, in_=wk_d.ap().rearrange("(t p) f -> p t f", p=P))
            nc.sync.dma_start(out=wq, in_=wq_d.ap().rearrange("(t p) f -> p t f", p=P))

            # ---- x^T (chunk-major; chunk 0 per-e-tile for earliest start) -
            xT_r = xT_d.ap().rearrange("(t p) s -> p t s", p=P)
            xts = xpool.tile([P, NE, S], F16, name="xts")
            for et in range(NE):
                nc.sync.dma_start(out=xts[:, et, 0:QC], in_=xT_r[:, et, 0:QC])
            nc.sync.dma_start(out=wv, in_=wv_d.ap().rearrange("(t p) f -> p t f", p=P))
            nc.sync.dma_start(out=ident, in_=id_d.ap())
            for cq in range(1, NQ):
                csl = slice(cq * QC, (cq + 1) * QC)
                nc.sync.dma_start(out=xts[:, :, csl], in_=xT_r[:, :, csl])
            nc.sync.dma_start(out=wo, in_=wo_d.ap().rearrange("(t p) g -> p t g", p=P))

            kts = [qkpool.tile([P, S], F16, name=f"kt{ft}", tag=f"kt{ft}") for ft in range(FT)]
            qts = [qkpool.tile([P, S], F16, name=f"qt{ft}", tag=f"qt{ft}") for ft in range(FT)]
            ots = [opool.tile([P, S], F16, name=f"ot{ft}", tag=f"ot{ft}") for ft in range(FT)]
            v_tiles = [
                vpool.tile([P, HPC, D + 1], F16, name=f"v{st}", tag=f"v{st}")
                for st in range(NST)
            ]

            # ---- filler machinery: one PE-side unit per next() -----------
            fillers = deque()

            def pump(n):
                for _ in range(n):
                    while fillers:
                        try:
                            next(fillers[0])
                            break
                        except StopIteration:
                            fillers.popleft()
                    else:
                        return

            def drain_now(gen):
                for _ in gen:
                    pass

            # ---- projection unit generators ------------------------------
            def proj_qk_units(w_tile, dst, ft, cq, which):
                ps = popool.tile([P, QC], F32, name=f"ps_{which}", tag="po")
                csl = slice(cq * QC, (cq + 1) * QC)
                for et in range(NE):
                    nc.tensor.matmul(
                        ps,
                        w_tile[:, et, ft * P : (ft + 1) * P],
                        xts[:, et, csl],
                        start=(et == 0),
                        stop=(et == NE - 1),
                    )
                    yield
                nc.vector.tensor_copy(dst[ft][:, csl], ps)
                yield

            def proj_v_units(st):
                vt = v_tiles[st]
                nc.vector.memset(vt[:, :, D : D + 1], 1.0)
                ps_v = popool.tile([P, FPC], F32, name="ps_v", tag="po")
                for et in range(NE):
                    nc.tensor.matmul(
                        ps_v,
                        xts[:, et, st * P : (st + 1) * P],
                        wv[:, et, :],
                        start=(et == 0),
                        stop=(et == NE - 1),
                    )
                    yield
                nc.vector.tensor_copy(
                    vt[:, :, 0:D], ps_v.rearrange("p (h d) -> p h d", d=D)
                )
                yield

            # ---- attention inner loop ------------------------------------
            def attn_core(pair, cq, per_kt=2):
                """Heads 2*pair, 2*pair+1 for query chunk cq. Returns the two
                [P, NQT, P] psum accumulators (cols 0..63 = sum(exp*v), col 64
                = sum(exp) per query-on-partition)."""
                csl = slice(cq * QC, (cq + 1) * QC)
                oacc = [
                    oaccpool.tile([P, NQT, P], F32, name=f"oacc{s}", tag="oacc")
                    for s in range(2)
                ]
                for kt in range(NST):
                    et_t = epool.tile([P, 2, QC], F16, name="et_t", tag="et_t")
                    ps_s = pspool.tile([P, 2, QC], F32, name="ps_s", tag="ps_s")
                    for sub in range(2):
                        lo, hi = sub * D, (sub + 1) * D
                        nc.tensor.matmul(
                            ps_s[:, sub, :],
                            kts[pair][lo:hi, kt * P : (kt + 1) * P],
                            qts[pair][lo:hi, csl],
                            start=True,
                            stop=True,
                        )
                    nc.scalar.activation(
                        out=et_t,
                        in_=ps_s,
                        func=mybir.ActivationFunctionType.Exp,
                        scale=SM,
                    )
                    # One accumulation group per oacc tile (= one PSUM bank):
                    # start pending-zeroes the whole 2KB zero region, so only
                    # the first slice write may carry start; later qt slices'
                    # first writes land on pending-zero bytes (read-as-zero).
                    for sub in range(2):
                        for qt in range(NQT):
                            nc.tensor.matmul(
                                oacc[sub][:, qt, 0 : D + 1],
                                et_t[:, sub, qt * P : (qt + 1) * P],
                                v_tiles[kt][:, 2 * pair + sub, :],
                                start=(kt == 0 and qt == 0),
                                stop=(kt == NST - 1 and qt == NQT - 1),
                            )
                    if kt > 0:
                        pump(per_kt)
                return oacc

            def finish_units(pair, cq, oacc, tail=False):
                """Normalize (fused psum evacuation) + transpose to [f, s]."""
                rec = spool.tile([P, 2, NQT], F32, name="rec", tag="rec")
                o_sb = spool.tile([P, NQT, P], F16, name="o_sb", tag="o_sb")
                for sub in range(2):
                    nc.vector.reciprocal(rec[:, sub, :], oacc[sub][:, :, D : D + 1])
                    yield
                for qt in range(NQT):
                    for sub in range(2):
                        nc.vector.tensor_scalar_mul(
                            o_sb[:, qt, sub * D : (sub + 1) * D],
                            oacc[sub][:, qt, 0:D],
                            rec[:, sub, qt : qt + 1],
                        )
                        yield
                pt = popool.tile([P, QC], F16, name="pt", tag="po")
                for qt in range(NQT):
                    nc.tensor.transpose(
                        pt[:, qt * P : (qt + 1) * P], o_sb[:, qt, :], ident
                    )
                    yield
                csl = slice(cq * QC, (cq + 1) * QC)
                if tail:
                    nc.scalar.activation(
                        out=ots[pair][:, csl],
                        in_=pt,
                        func=mybir.ActivationFunctionType.Copy,
                    )
                else:
                    nc.vector.tensor_copy(ots[pair][:, csl], pt)
                yield

            def outproj_units(cq, tail=False):
                """Output projection for chunk cq (needs both pairs' ots)."""
                for sti in range(NQT):
                    st = cq * NQT + sti
                    out_sb = outpool.tile([P, E], F16, name="out_sb", tag="out_sb")
                    for gc in range(2):
                        ps_out = popool.tile([P, QC], F32, name="ps_out", tag="po")
                        for ft in range(FT):
                            nc.tensor.matmul(
                                ps_out,
                                ots[ft][:, st * P : (st + 1) * P],
                                wo[:, ft, gc * QC : (gc + 1) * QC],
                                start=(ft == 0),
                                stop=(ft == FT - 1),
                            )
                            yield
                        if tail:
                            nc.scalar.activation(
                                out=out_sb[:, gc * QC : (gc + 1) * QC],
                                in_=ps_out,
                                func=mybir.ActivationFunctionType.Copy,
                            )
                        else:
                            nc.vector.tensor_copy(
                                out_sb[:, gc * QC : (gc + 1) * QC], ps_out
                            )
                        yield
                    nc.sync.dma_start(
                        out=out_d.ap()[st * P : (st + 1) * P, :], in_=out_sb
                    )

            # ---- emission (= priority) -----------------------------------
            # startup: chunk-0 projections inline, rest as deadline-ordered
            # fillers (k before v before q; pair-1 weights later; outproj
            # appended as it becomes available).
            drain_now(proj_qk_units(wk, kts, 0, 0, "k0"))
            drain_now(proj_qk_units(wq, qts, 0, 0, "q0"))
            for st in range(NQT):
                drain_now(proj_v_units(st))

            for cq in range(1, NQ):
                fillers.append(proj_qk_units(wk, kts, 0, cq, "k0"))
                for st in range(NQT * cq, NQT * (cq + 1)):
                    fillers.append(proj_v_units(st))
                fillers.append(proj_qk_units(wq, qts, 0, cq, "q0"))
            for cq in range(NQ):
                fillers.append(proj_qk_units(wk, kts, 1, cq, "k1"))
                fillers.append(proj_qk_units(wq, qts, 1, cq, "q1"))

            for pair in range(2):
                for cq in range(NQ):
                    last = pair == 1 and cq == NQ - 1
                    oacc = attn_core(pair, cq, per_kt=4)
                    fin = finish_units(pair, cq, oacc, tail=last)
                    if last:
                        drain_now(fin)
                        drain_now(outproj_units(cq, tail=True))
                    else:
                        fillers.appendleft(fin)
                        if pair == 1:
                            fillers.append(outproj_units(cq))
            while fillers:
                pump(64)

    nc.compile()
    return nc


_NC_CACHE = None


def _get_nc():
    global _NC_CACHE
    if _NC_CACHE is None:
        _NC_CACHE = _build()
    return _NC_CACHE


def make_in_maps(x, Wq, Wk, Wv, Wo):
    in_maps = []
    xTs = [np.ascontiguousarray(x[b].T, dtype=np.float16) for b in range(B)]
    ident = np.eye(P, dtype=np.float16)
    for c in range(NCORES):
        b, hg = c // GPB, c % GPB
        fsl = slice(hg * FPC, (hg + 1) * FPC)
        in_maps.append({
            "xT": xTs[b],
            "wqT": np.ascontiguousarray(Wq[fsl, :].T, dtype=np.float16),
            "wkT": np.ascontiguousarray(Wk[fsl, :].T, dtype=np.float16),
            "wvT": np.ascontiguousarray(Wv[fsl, :].T, dtype=np.float16),
            "woT": np.ascontiguousarray(Wo[:, fsl].T, dtype=np.float16),
            "ident": ident,
        })
    return in_maps


def kernel(x, Wq, bq, Wk, bk, Wv, bv, Wo, bo):
    x = np.asarray(x, dtype=np.float32)
    Wq, Wk, Wv, Wo = (np.asarray(a, dtype=np.float32) for a in (Wq, Wk, Wv, Wo))
    bq, bk, bv, bo = (np.asarray(a, dtype=np.float32) for a in (bq, bk, bv, bo))
    if np.any(bq) or np.any(bk) or np.any(bv):
        # projection biases are zero for this problem spec; folding nonzero
        # biases into an augmented input row is not implemented.
        raise NotImplementedError("nonzero projection biases not supported")

    nc = _get_nc()
    in_maps = make_in_maps(x, Wq, Wk, Wv, Wo)
    res = run_bass_kernel_spmd(nc, in_maps, core_ids=list(range(NCORES)))
    out = np.empty((B, S, E), dtype=np.float32)
    for b in range(B):
        acc = res.results[b * GPB]["out"].astype(np.float32)
        for hg in range(1, GPB):
            acc += res.results[b * GPB + hg]["out"].astype(np.float32)
        out[b] = acc
    out += bo[None, None, :]
    return out
